# revision 68
# baseline (speedup 1.0000x reference)
"""
BasicCrossAttention Trainium2 kernel (8 NeuronCores, SPMD head-parallel).

Sharding: 16 heads split across 8 cores (2 heads/core).  Each core computes
Q/K/V projections for its 2 heads (column-sharded Wq/Wk/Wv), per-head QK
LayerNorm, full cross-attention over B*H_local, and a row-sharded partial of
the output projection.  The host sums the 8 fp16 partial outputs and adds
the bias.

Device math is bf16 matmuls with fp32 PSUM accumulation.

Host-side prep: x1/x2 are transposed to [B, DM, N] (so the device only does
plain contiguous DMAs -- DMA-xbar transposes serialize ~7us apiece on the
sync queue), and the K/Q weights arrive mean-centered per head (LN mean
folded into the weights; the device only needs E[x^2]).

Structure (per core):
  - K/Q projections are weight-stationary (lhsT = W chunk), producing kT/qT
    [feat, rows] directly -- no PE transposes, LDWEIGHTS hidden under N=512
    matmuls.  LN rstd: block-diagonal ones-matmul sums y^2 over each head's
    64 partitions (output pre-replicated across the head), then
    rstd = exp(-0.5*ln(var+eps)) on ACT; per-partition ln_g folds into the
    drain scalar_tensor_tensor.  V stays x-stationary ([rows, feat] for the
    AV lhsT), 4 row-tiles packed per PSUM bank.
  - Attention in S^T layout [m,n]: head-pair row-packed score matmuls (K=64
    at row groups 0/64, truly concurrent in the PE sub-arrays) into one
    [128,1024] PSUM pair, one paired exp, and V-augmented-with-ones AV
    matmuls giving the softmax normalizer for free.
  - Softmax normalizer: 1/Z = exp(-ln Z) on ACT (Ln+Exp share the
    natural_log_exp_and_others table set with softmax's exp -- the build
    pins all ACT functions there, else the table-load inserter ping-pongs
    at 1.3us per reload).  Broadcast + second-head multiply on GPSIMD,
    first-head multiply on DVE.
  - ALL engine queues are strictly in-order, so every PSUM tile is drained
    to SBUF by one fast copy (ysb, av_sb on ACT behind the chunk's last
    exp) and the slow stats/normalizer chains read the SBUF copy; a PSUM
    tile whose release waits on a multi-us chain FIFO-blocks the whole PE
    queue at the next allocation.
  - Output projection partials drain as fp16; outp(0)+outp(1) interleave
    into attn(1) chunk-by-chunk with a burst at each chunk boundary (where
    the next chunk's scores wait on the lagging exp stream).
  - PSUM budget (8 banks): st 2x2 (scores double-buffer) + y 2x1
    (K/Q/V/var/fps rotation) + av 1x2.
Emission order IS the dependency order; attn(0) starts 9 prod-yields in,
prod(1) paces into the rest of attn(0) (away from chunk boundaries).
GPSIMD runs a single custom-op library (PartitionBroadcast) -- mixing in
MULTIPLY/MEMSET forces LIBRARY_RELOADs costing ~6us of hidden load latency
at the queue head.  Phase balancing: attn(0) is PE-bound (prod(1) rides
inside it) while attn(1) is ACT-bound, so prod(1)'s Q-rg1 pass (first
read by attn(1) chunk 2) is deferred into attn(1) as PE filler.
Measured 240.6us on HW (traced), rel err 5.6e-3 (baseline at session
start: 395us / 381us graded).
"""

import os
import sys

for _p in ("/root/.axon_site", "/root/.axon_site/_ro/trn_rl_repo",
           "/root/.axon_site/_ro/pypackages", "/opt/trn_rl_repo"):
    if os.path.isdir(_p) and _p not in sys.path:
        sys.path.append(_p)

import numpy as np
import ml_dtypes
from contextlib import ExitStack

B = 2
N = 2048          # query rows (x1)
M = 2048          # key rows (x2)
DM = 1024         # d_model
H = 16            # total heads
HD = 64           # head dim
NCORES = 8
HL = H // NCORES  # heads per core = 2
LOC = HL * HD     # local feature width = 128
SCALE = 8.0 / HD  # mup scale
EPS = 1e-5

_COMPILED = None          # cached Bass program
LAST_RESULT = None        # BassKernelResults of last run (for profiling)
DEBUG_DUMPS = False       # set True (before _build) to emit intermediate dumps

_SENTINEL = object()


def _emit(ctx, tc, aps, with_b):
    import concourse.bass as bass
    from concourse import mybir

    nc = tc.nc
    f32 = mybir.dt.float32
    f16 = mybir.dt.float16
    bf16 = mybir.dt.bfloat16
    AF = mybir.ActivationFunctionType
    OP = mybir.AluOpType

    x1, x2, wqT, wkT, wvT, wp, ln_g, ln_b, out = (
        aps["x1"], aps["x2"], aps["wqT"], aps["wkT"], aps["wvT"],
        aps["wp"], aps["ln_g"], aps["ln_b"], aps["out"])

    const = ctx.enter_context(tc.tile_pool(name="const", bufs=1))
    xT_pool = ctx.enter_context(tc.tile_pool(name="xTp", bufs=4))
    nat_pool = ctx.enter_context(tc.tile_pool(name="natp", bufs=10))
    big_pool = ctx.enter_context(tc.tile_pool(name="bigp", bufs=2))
    # Deeper SBUF rotations absorb the ACT/PE jitter at chunk boundaries:
    # pT=4 lets the exp stream run a full extra mc ahead of the AV matmuls,
    # dr=3 decouples consecutive chunks' normalizer chains, osb=6 deepens
    # the outp drain pipeline at the tail.
    pT_pool = ctx.enter_context(tc.tile_pool(name="pTp", bufs=4))
    out_pool = ctx.enter_context(tc.tile_pool(name="outp", bufs=6))
    dr_pool = ctx.enter_context(tc.tile_pool(name="drp", bufs=3))
    ps_st = ctx.enter_context(tc.tile_pool(name="psst", bufs=2, space="PSUM"))
    ps_y = ctx.enter_context(tc.tile_pool(name="psy", bufs=2, space="PSUM"))
    ps_av = ctx.enter_context(tc.tile_pool(name="psav", bufs=1, space="PSUM"))

    # ---------------- constants / weights ----------------
    # [in 128, fc 8, feat 128]; wk/wq serve as lhsT (weight-stationary
    # projections), wv as rhs (x-stationary, natural [rows, feat] output).
    # Weight DMA dispatches spread over three queues: serialized on one
    # queue they delay the pass-0 xT loads behind them (~1us DIRECT2D
    # dispatch each).
    wk_sb = const.tile([128, 8, LOC], bf16)
    nc.sync.dma_start(wk_sb, wkT.rearrange("(a p) o -> p a o", p=128))
    wv_sb = const.tile([128, 8, LOC], bf16)
    nc.gpsimd.dma_start(wv_sb, wvT.rearrange("(a p) o -> p a o", p=128))
    wq_sb = const.tile([128, 8, LOC], bf16)
    nc.sync.dma_start(wq_sb, wqT.rearrange("(a p) o -> p a o", p=128))
    wp_sb = const.tile([128, DM], bf16)
    nc.gpsimd.dma_start(wp_sb, wp)

    # Block-diagonal head replicator: ones_blk[p, j] = 1 iff head(p)==head(j).
    # matmul(lhsT=ones_blk, rhs=y^2) -> per-head sum over d, already
    # broadcast across that head's 64 partitions.
    ones_blk = const.tile([128, 128], f16)
    nc.gpsimd.memset(ones_blk, 0.0)
    nc.gpsimd.memset(ones_blk[0:HD, 0:HD], 1.0)
    nc.gpsimd.memset(ones_blk[HD:128, HD:128], 1.0)
    eps_col = const.tile([128, 1], f32)
    nc.gpsimd.memset(eps_col, EPS)

    # ln params replicated per local feature: partition p <- param[p % 64]
    g_col = const.tile([128, 1], f32)
    nc.gpsimd.dma_start(g_col, bass.AP(tensor=ln_g.tensor, offset=ln_g.offset,
                                       ap=[[0, HL], [1, HD]]))
    b_col = const.tile([128, 1], f32)
    nc.gpsimd.dma_start(b_col, bass.AP(tensor=ln_b.tensor, offset=ln_b.offset,
                                       ap=[[0, HL], [1, HD]]))
    gq_col = const.tile([128, 1], f32)
    nc.vector.tensor_scalar_mul(gq_col, g_col, SCALE)
    bq_col = const.tile([128, 1], f32)
    nc.vector.tensor_scalar_mul(bq_col, b_col, SCALE)

    # K and Q weight head-blocks arrive mean-centered from the host (LN mean
    # folded into the weights; only E[x^2] needed per tile).

    # persistent per-batch tiles (bufs=2 -> both batches in flight)
    kT = [None, None]
    qT = [None, None]
    Vt = [None, None]
    hoT = [None, None]

    # ---------------- phase generators ----------------
    def prod(b):
        """Project K|V (from x2) and Q (from x1) for batch b, with QK-LN.

        K/Q are weight-stationary (lhsT = W chunk), producing [feat, rows]
        directly in kT/qT orientation -- no PE transposes, and LDWEIGHTS
        (128 cols) hides under the N=512 matmuls.  LN per (head, token) in
        this orientation: y^2 summed over d via a block-diagonal ones
        matmul whose output is already replicated across each head's 64
        partitions, then rstd = exp(-0.5*ln(var+eps)) on ACT, folded into
        the PSUM drain together with per-partition ln_g.
        V stays x-stationary ([rows, feat] output for the AV lhsT); 4
        row-tiles pack into one PSUM bank as separate accumulation groups.
        """
        kT[b] = big_pool.tile([128, M], bf16, tag="kT", name=f"kT{b}")
        qT[b] = big_pool.tile([128, N], bf16, tag="qT", name=f"qT{b}")
        Vt[b] = big_pool.tile([128, 16, 2 * (HD + 1)], bf16, tag="V",
                              name=f"V{b}")
        # All 32 softmax-normalizer ones columns in ONE strided memset: as
        # 32 separate 50ns memsets they sit in the DVE queue ahead of the
        # startup stats-chain copies and stretch the lead-in.
        nc.vector.memset(
            bass.AP(tensor=Vt[b].tensor, offset=Vt[b].offset + HD,
                    ap=[Vt[b].ap[0], [2 * (HD + 1), 16], [HD + 1, 2]]),
            1.0)
        # Q-rg0 first: attention chunk 0 needs qT[:, 0:512] plus the first
        # kT tiles, so this order lets the exp stream start early.
        for pidx, (src, is_q, rg) in enumerate(((x1, True, 0), (x2, False, 0),
                                                (x2, False, 1), (x1, True, 1))):
            solo = (b == 0 and pidx < 2)
            dst = qT[b] if is_q else kT[b]
            gc = gq_col if is_q else g_col
            w_sb = wq_sb if is_q else wk_sb
            # x arrives pre-transposed from the host ([B, DM, N]) so the
            # xT tiles are plain contiguous DMAs (DMA-xbar transposes
            # serialized ~7us apiece on the sync queue and gated startup).
            xT = xT_pool.tile([128, 8, 1024], bf16, tag="xT", bufs=4,
                              name=f"xT{b}{int(is_q)}{rg}")
            for fc in range(8):
                eng = nc.sync if fc % 2 == 0 else nc.gpsimd
                eng.dma_start(
                    out=xT[:, fc, :],
                    in_=src[b, fc * 128:(fc + 1) * 128,
                            rg * 1024:(rg + 1) * 1024])
            yield
            for rc2 in range(2):
                gc512 = rg * 2 + rc2          # global 512-row chunk 0..3
                rsl = slice(rc2 * 512, (rc2 + 1) * 512)
                dsl = slice(gc512 * 512, (gc512 + 1) * 512)
                yps = ps_y.tile([128, 512], f32, tag="y", bufs=2,
                                name=f"y{b}{int(is_q)}{gc512}")
                for fc in range(8):
                    nc.tensor.matmul(yps, lhsT=w_sb[:, fc, :],
                                     rhs=xT[:, fc, rsl],
                                     start=(fc == 0), stop=(fc == 7))
                # f16 copy first: the y PSUM bank releases after this one
                # ~0.7us op, so the next projection matmul queued on PE is
                # never FIFO-blocked behind the multi-us stats chain (the
                # engine queues are strictly in-order -- a stalled MM at the
                # head of the PE queue stalls ALL later attention matmuls).
                # Everything downstream reads the SBUF copy.
                ysb = nat_pool.tile([128, 512], f16, tag="ysb", bufs=3)
                if solo:
                    nc.scalar.copy(ysb, yps)
                else:
                    nc.vector.tensor_copy(ysb, yps)
                sq = nat_pool.tile([128, 512], f16, tag="sq", bufs=3)
                nc.vector.tensor_mul(sq, ysb, ysb)
                yield
                # per-(head, token) sum of y^2 over d, replicated across the
                # head's partitions by the block-diagonal ones matmul; var
                # shares the "y" slots (1 bank, released by the Ln read).
                # During the solo lead-in the attention "st" banks are idle;
                # parking var there keeps the 2-slot "y" rotation from
                # serializing the startup projection chunks.
                var_pool, var_tag = (ps_st, "st") if solo else (ps_y, "y")
                var = var_pool.tile([128, 512], f32, tag=var_tag,
                                    name=f"var{b}{int(is_q)}{gc512}")
                nc.tensor.matmul(var, lhsT=ones_blk, rhs=sq,
                                 start=True, stop=True)
                lnv = nat_pool.tile([128, 512], f32, tag="lnv", bufs=3)
                nc.scalar.activation(lnv, var, AF.Ln, bias=eps_col,
                                     scale=1.0 / HD)
                rstd = nat_pool.tile([128, 512], bf16, tag="rstd", bufs=3)
                nc.scalar.activation(rstd, lnv, AF.Exp, scale=-0.5)
                # drain: dst = (y * g) * rstd  (ln_b == 0 fast path; the
                # general-b build adds a per-partition b_col afterwards)
                nc.vector.scalar_tensor_tensor(dst[:, dsl], ysb, gc, rstd,
                                               op0=OP.mult, op1=OP.mult)
                if with_b:
                    bc = bq_col if is_q else b_col
                    nc.vector.tensor_scalar_add(dst[:, dsl], dst[:, dsl], bc)
                yield
                if not is_q:
                    vps = ps_y.tile([128, 4, 128], f32, tag="y", bufs=2,
                                    name=f"v{b}{gc512}")
                    for mi4 in range(4):
                        xsl = slice(rc2 * 512 + mi4 * 128,
                                    rc2 * 512 + (mi4 + 1) * 128)
                        for fc in range(8):
                            nc.tensor.matmul(vps[:, mi4, :],
                                             lhsT=xT[:, fc, xsl],
                                             rhs=wv_sb[:, fc, :],
                                             start=(fc == 0), stop=(fc == 7),
                                             skip_group_check=True)
                    for mi4 in range(4):
                        mt = gc512 * 4 + mi4
                        vt = Vt[b][:, mt, :]
                        vt3 = bass.AP(tensor=vt.tensor, offset=vt.offset,
                                      ap=[vt.ap[0], [HD + 1, HL], [1, HD]])
                        vsrc = vps[:, mi4, :].rearrange(
                            "p (h x) -> p h x", h=HL)
                        if solo and mi4 % 2 == 1:
                            nc.scalar.copy(vt3, vsrc)
                        else:
                            nc.vector.tensor_copy(vt3, vsrc)
                    yield

    def attn(b):
        """S^T -> exp -> (V|1)^T @ P^T, head-pair packed.

        The mc pipeline is skewed: scores lead exp by 1 and AV by 2 mc
        steps.  Engine queues are in-order, so at a chunk boundary the PE
        FIFO then holds ready score matmuls of the next chunk while the
        previous chunk's exp/drain chains catch up -- AV(mc0) of the next
        chunk only enters the FIFO two steps later, by which time the av
        PSUM tile (bufs=1) has been released by the av_sb copy."""
        hoT[b] = big_pool.tile([128, N], bf16, tag="hoT", name=f"hoT{b}")
        for nc4 in range(4):  # 512-wide query column chunks
            ns = slice(nc4 * 512, (nc4 + 1) * 512)
            av = ps_av.tile([128, 1024], f32, tag="av", bufs=1,
                            name=f"av{b}{nc4}")
            sts = {}
            pTs = {}

            def emit_scores(mc):
                mcs = slice(mc * 128, (mc + 1) * 128)
                st = ps_st.tile([128, 1024], f32, tag="st",
                                name=f"st{b}{nc4}{mc}")
                for h in range(HL):
                    nc.tensor.matmul(st[:, h * 512:(h + 1) * 512],
                                     lhsT=kT[b][h * HD:(h + 1) * HD, mcs],
                                     rhs=qT[b][h * HD:(h + 1) * HD, ns],
                                     start=True, stop=True)
                sts[mc] = st

            def emit_exp(mc):
                pT = pT_pool.tile([128, 1024], bf16, tag="pT")
                nc.scalar.activation(pT, sts.pop(mc), AF.Exp)
                pTs[mc] = pT

            def emit_av(mc):
                pT = pTs.pop(mc)
                for h in range(HL):
                    nc.tensor.matmul(
                        av[0:HD + 1, h * 512:(h + 1) * 512],
                        lhsT=Vt[b][:, mc, h * (HD + 1):(h + 1) * (HD + 1)],
                        rhs=pT[:, h * 512:(h + 1) * 512],
                        start=(mc == 0), stop=(mc == 15),
                        skip_group_check=True)

            for mc in range(16):
                emit_scores(mc)
                if mc >= 1:
                    emit_exp(mc - 1)
                if mc >= 2:
                    emit_av(mc - 2)
                yield
            emit_exp(15)
            emit_av(14)
            emit_av(15)
            # drain: raw AV + normalizer -> normalized hoT chunk.
            # Both inputs of every tensor_tensor op must share a start
            # partition (BIR verifier); single-src ops (reciprocal,
            # activation, partition_broadcast) may cross partitions, so the
            # Z-row work lands on partition 0 and downstream stays 0-based.
            # One f16 copy (incl. the Z row; f16 keeps ~0.05% precision)
            # releases the av PSUM tile in <1us so the next chunk's AV
            # matmuls never stall (keeps PE HAM-warm); the whole normalizer
            # chain then runs from SBUF off the PE critical path.  1/Z on
            # DVE InstReciprocal (~6.5us single-lane, but ACT is the
            # saturated engine mid-kernel); the last chunk of each batch
            # uses ACT exp(-ln Z) instead, since by then the exp stream is
            # done and the shorter ACT chain trims the kernel tail.
            # One f16 copy (incl. the Z row) releases the av PSUM tile, on
            # ACT: it sits right behind the chunk's last exp in the ACT
            # FIFO and so runs deterministically ~1.2us after the last AV
            # matmul.  On DVE it can queue behind osb copies that wait on
            # PE, which waits on av -- a ~7us serialization per chunk.
            av_sb = dr_pool.tile([128, 1024], f16, tag="avsb")
            lnz = dr_pool.tile([128, 1024], f32, tag="lnz")
            if nc4 == 3:
                # Last chunk of the batch: nothing reuses av next, so Ln
                # reads the Z row straight from PSUM (drops the copy leg
                # from the tail-critical chain) while DVE copies the AV
                # rows in parallel.
                nc.vector.tensor_copy(av_sb[0:HD, :], av[0:HD, :])
                nc.scalar.activation(lnz[0:1, :], av[HD:HD + 1, :], AF.Ln)
            else:
                # 1/Z = exp(-ln(Z)) on ACT; Ln+Exp live in the same table
                # set as softmax's Exp (natural_log_exp_and_others) -> no
                # reload.  (A DVE InstReciprocal here measures 6.5us
                # single-lane and, because engine queues are in-order,
                # stalls every later DVE/gpsimd op behind it.)
                nc.vector.tensor_copy(av_sb[0:HD + 1, :], av[0:HD + 1, :])
                nc.scalar.activation(lnz[0:1, :], av_sb[HD:HD + 1, :], AF.Ln)
            yield
            rz = dr_pool.tile([128, 1024], f32, tag="rz")
            nc.scalar.activation(rz[0:1, :], lnz[0:1, :], AF.Exp, scale=-1.0)
            bcast = dr_pool.tile([128, 1024], f32, tag="bc")
            if nc4 == 3:
                # Tail-critical: 4-way split so the first half-chunk's
                # multiplies (and with them the first outp units) start a
                # broadcast earlier.
                for half in range(2):
                    hs = slice(half * 256, half * 256 + 256)
                    h1s = slice(512 + half * 256, 512 + half * 256 + 256)
                    nsh = slice(nc4 * 512 + half * 256,
                                nc4 * 512 + half * 256 + 256)
                    nc.gpsimd.partition_broadcast(bcast[0:HD, hs],
                                                  rz[0:1, hs])
                    nc.gpsimd.partition_broadcast(bcast[0:HD, h1s],
                                                  rz[0:1, h1s])
                    nc.vector.tensor_mul(hoT[b][0:HD, nsh],
                                         av_sb[0:HD, hs], bcast[0:HD, hs])
                    nc.vector.tensor_mul(hoT[b][HD:128, nsh],
                                         av_sb[0:HD, h1s], bcast[0:HD, h1s])
            else:
                nc.gpsimd.partition_broadcast(bcast[0:HD, 0:512],
                                              rz[0:1, 0:512])
                nc.gpsimd.partition_broadcast(bcast[0:HD, 512:1024],
                                              rz[0:1, 512:1024])
            # Both hoT multiplies on DVE: putting one on gpsimd alternates
            # its DSP library with PartitionBroadcast's, and each
            # LIBRARY_RELOAD costs ~6us of hidden load latency at the
            # queue head.
            if nc4 != 3:
                nc.vector.tensor_mul(hoT[b][0:HD, ns], av_sb[0:HD, 0:512],
                                     bcast[0:HD, 0:512])
                nc.vector.tensor_mul(hoT[b][HD:128, ns],
                                     av_sb[0:HD, 512:1024],
                                     bcast[0:HD, 512:1024])
            if DEBUG_DUMPS and b == 0 and nc4 in (0, 1):
                nc.sync.dma_start(aps[f"dbg{nc4}_avsb"], av_sb)
                nc.sync.dma_start(aps[f"dbg{nc4}_rz"], rz)
                nc.sync.dma_start(aps[f"dbg{nc4}_bc"], bcast)
                nc.sync.dma_start(aps[f"dbg{nc4}_ho"], hoT[b][:, ns])
            if DEBUG_DUMPS and b == 0 and nc4 == 3:
                nc.sync.dma_start(aps["dbg_kT"], kT[0])
                nc.sync.dma_start(aps["dbg_qT"], qT[0])
                nc.sync.dma_start(
                    aps["dbg_Vt"], Vt[0].rearrange("p a b -> p (a b)"))
            yield

    def outp_unit(b, nt, oc, tail=False):
        """One output-projection tile: matmul + fp16 drain + store.

        fps reuses the prod-phase "y" PSUM slots (same 1-bank size, and the
        last y-tag use — prod(1) — is fully emitted before the first outp
        unit), keeping total PSUM at 8 banks.  Tail units (after the exp
        stream has finished) drain on ACT, doubling the 2-slot fps/osb
        pipeline rate that otherwise sets the kernel tail."""
        fps = ps_y.tile([128, 512], f32, tag="y", bufs=2,
                        name=f"fps{b}{nt}{oc}")
        nc.tensor.matmul(fps,
                         lhsT=hoT[b][:, nt * 128:(nt + 1) * 128],
                         rhs=wp_sb[:, oc * 512:(oc + 1) * 512],
                         start=True, stop=True)
        osb = out_pool.tile([128, 512], f16, tag="osb")
        if tail:
            nc.scalar.copy(osb, fps)
        else:
            nc.vector.tensor_copy(osb, fps)
        nc.sync.dma_start(
            out[b, nt * 128:(nt + 1) * 128, oc * 512:(oc + 1) * 512],
            osb)

    def run_all(g):
        for _ in g:
            pass

    def run_n(g, n):
        for _ in range(n):
            if next(g, _SENTINEL) is _SENTINEL:
                return False
        return True

    def interleave(ga, gb, ka, kb):
        """Alternate ka steps of ga with kb steps of gb until both drain."""
        alive_a, alive_b = True, True
        while alive_a or alive_b:
            for _ in range(ka):
                if alive_a:
                    alive_a = next(ga, _SENTINEL) is not _SENTINEL
            for _ in range(kb):
                if alive_b:
                    alive_b = next(gb, _SENTINEL) is not _SENTINEL

    def attn0_with_prod1(ga, start_step):
        """Finish attn(0) with prod(1) paced in.  Prod yields are kept away
        from chunk boundaries: a prod matmul whose PSUM slot is still held
        (stats chain in flight) at the head of the in-order PE queue would
        block the next chunk's ready score matmuls queued behind it."""
        gp = prod(1)
        alive = True
        step = start_step
        fed = 0
        while alive:
            alive = next(ga, _SENTINEL) is not _SENTINEL
            step += 1
            # Feed only the first 19 prod(1) yields (through KV-rg1) here;
            # Q-rg1 is deferred into attn(1), whose ACT-bound chunks need
            # the PE filler (attn(0) is PE-bound and sheds it gladly) --
            # qT[1] cols 1024: are first read by attn(1) chunk 2.
            if fed < 19:
                ph = step % 18
                n = 0 if (ph >= 15 or ph < 2) else (
                    2 if ph == 8 else (1 if step % 3 == 0 else 0))
                for _ in range(n):
                    if fed < 19:
                        if next(gp, _SENTINEL) is _SENTINEL:
                            fed = 19
                        else:
                            fed += 1
        return gp

    def attn1_with_outp(gp_rest):
        """attn(1) with outp(0) and outp(1) interleaved chunk-by-chunk.

        outp(0)'s hoT is fully written before this phase; outp(1) chunks are
        appended to the work queue as attn(1) finishes each 512-column chunk.
        At each chunk boundary a burst of up to 6 units is emitted -- the
        next chunk's first scores wait on the lagging exp stream, so the
        boundary is exactly where the PE queue needs pre-emitted filler."""
        units = [(0, nt, oc) for nt in range(16) for oc in range(2)]
        ga = attn(1)
        step = 0
        nc4_done = 0
        alive = True
        gp_alive = gp_rest is not None
        while alive or units:
            if alive:
                alive = next(ga, _SENTINEL) is not _SENTINEL
                step += 1
                # deferred Q-rg1 of prod(1): 1 yield per 3 steps, away from
                # chunk boundaries (5 yields total, done within chunk 1)
                if gp_alive and step % 3 == 1 and 2 <= step % 18 < 15:
                    gp_alive = next(gp_rest, _SENTINEL) is not _SENTINEL
                # 18 yields per nc4 chunk (16 mc + 2 drain)
                if alive and step % 18 == 0:
                    for nt in range(nc4_done * 4, nc4_done * 4 + 4):
                        for oc in range(2):
                            units.append((1, nt, oc))
                    nc4_done += 1
                if not alive:
                    while nc4_done < 4:
                        for nt in range(nc4_done * 4, nc4_done * 4 + 4):
                            for oc in range(2):
                                units.append((1, nt, oc))
                        nc4_done += 1
            # Skip pops on the first two yields of each chunk: a unit whose
            # hoT slice is still in the drain chain would FIFO-block the
            # next chunk's ready score matmuls behind it on PE.  Draining
            # eagerly (2 pops while the queue is deep) measures better than
            # back-loading: leftovers past the last chunk stretch the tail.
            if units and (not alive or step % 18 >= 2):
                outp_unit(*units.pop(0), tail=not alive and len(units) % 2 == 0)
                if len(units) > 12 and step % 18 >= 2:
                    outp_unit(*units.pop(0))

    # Emission IS the dependency order (Tile tracks emission-ordered deps)
    # and largely the execution order, so attn(0) must be emitted early to
    # start the exp stream early.  prod(0) passes 1-2 (Q-rg0: 5 yields,
    # KV-rg0: 7 yields) give attn chunk 0 its inputs for mc 0-7; chunk 0's
    # mc>=8 readers need pass 3 (KV-rg1, 7 yields), so attn paces 1:1
    # behind the rest of prod(0) (12 yields).
    gp0 = prod(0)
    run_n(gp0, 9)               # Q-rg0 + KV-rg0 through K/V-rc0
    ga0 = attn(0)
    for _ in range(15):
        run_n(ga0, 1)
        run_n(gp0, 1)
    run_all(gp0)                # safety drain (no-op when counts match)
    gp_rest = attn0_with_prod1(ga0, start_step=15)
    attn1_with_outp(gp_rest)


def _build(with_b=False):
    global _COMPILED
    if _COMPILED is not None and _COMPILED[0] == with_b:
        return _COMPILED[1]
    import concourse.tile as tile
    from concourse import bacc, mybir
    from concourse.hw_specs import get_activation_tables

    # Pin Exp/Ln/Copy/Identity/Square to the one table set that has them all
    # (natural_log_exp_and_others); otherwise the table-load inserter
    # ping-pongs between exp_and_others and the ln set (1.3us per reload,
    # on the softmax critical path).  Set ids are positional, so entries are
    # edited in place, never removed.
    _AF = mybir.ActivationFunctionType
    _tabs = get_activation_tables("gen3")
    for _name, _fns in _tabs.items():
        if _name != "natural_log_exp_and_others":
            for _f in (_AF.Exp, _AF.Ln, _AF.Copy, _AF.Identity, _AF.Square):
                _fns.discard(_f)

    nc = bacc.Bacc("TRN2", target_bir_lowering=False, debug=False,
                   enable_asserts=False)
    bf16 = mybir.dt.bfloat16
    f32 = mybir.dt.float32
    f16 = mybir.dt.float16
    aps = {
        "x1": nc.dram_tensor("x1", [B, DM, N], bf16, kind="ExternalInput").ap(),
        "x2": nc.dram_tensor("x2", [B, DM, M], bf16, kind="ExternalInput").ap(),
        "wqT": nc.dram_tensor("wqT", [DM, LOC], bf16, kind="ExternalInput").ap(),
        "wkT": nc.dram_tensor("wkT", [DM, LOC], bf16, kind="ExternalInput").ap(),
        "wvT": nc.dram_tensor("wvT", [DM, LOC], bf16, kind="ExternalInput").ap(),
        "wp": nc.dram_tensor("wp", [LOC, DM], bf16, kind="ExternalInput").ap(),
        "ln_g": nc.dram_tensor("ln_g", [HD], f32, kind="ExternalInput").ap(),
        "ln_b": nc.dram_tensor("ln_b", [HD], f32, kind="ExternalInput").ap(),
        "out": nc.dram_tensor("out", [B, N, DM], f16, kind="ExternalOutput").ap(),
    }
    if DEBUG_DUMPS:
        for c in (0, 1):
            aps[f"dbg{c}_avsb"] = nc.dram_tensor(
                f"dbg{c}_avsb", [128, 1024], bf16, kind="ExternalOutput").ap()
            aps[f"dbg{c}_rz"] = nc.dram_tensor(
                f"dbg{c}_rz", [128, 1024], f32, kind="ExternalOutput").ap()
            aps[f"dbg{c}_bc"] = nc.dram_tensor(
                f"dbg{c}_bc", [128, 1024], f32, kind="ExternalOutput").ap()
            aps[f"dbg{c}_ho"] = nc.dram_tensor(
                f"dbg{c}_ho", [128, 512], bf16, kind="ExternalOutput").ap()
        aps["dbg_kT"] = nc.dram_tensor(
            "dbg_kT", [128, M], bf16, kind="ExternalOutput").ap()
        aps["dbg_qT"] = nc.dram_tensor(
            "dbg_qT", [128, N], bf16, kind="ExternalOutput").ap()
        aps["dbg_Vt"] = nc.dram_tensor(
            "dbg_Vt", [128, 16 * 130], bf16, kind="ExternalOutput").ap()
    with tile.TileContext(nc) as tc, ExitStack() as ctx:
        _emit(ctx, tc, aps, with_b)
    nc.compile()
    _COMPILED = (with_b, nc)
    return nc


def kernel(x1, x2, Wq, Wk, Wv, Wp, bp, ln_g, ln_b):
    global LAST_RESULT
    from concourse.bass_utils import run_bass_kernel_spmd

    nc = _build(with_b=bool(np.any(np.asarray(ln_b, dtype=np.float32))))
    bf = ml_dtypes.bfloat16
    # Host-side transpose to [B, DM, N]: device consumes x only in
    # transposed form, and plain DMAs are ~6x faster than DMA-xbar
    # transposes on the sync queue.
    x1b = np.ascontiguousarray(
        np.asarray(x1, dtype=np.float32).transpose(0, 2, 1)).astype(bf)
    x2b = np.ascontiguousarray(
        np.asarray(x2, dtype=np.float32).transpose(0, 2, 1)).astype(bf)
    Wq = np.asarray(Wq, dtype=np.float32)
    Wk = np.asarray(Wk, dtype=np.float32)
    Wv = np.asarray(Wv, dtype=np.float32)
    Wp = np.asarray(Wp, dtype=np.float32)
    # Fold the LN mean into the K/Q weights: subtract each head's mean over
    # its 64 output features (torch Linear rows), so projections come out
    # zero-mean per head and the device only needs E[x^2].
    Wq = (Wq.reshape(H, HD, DM) -
          Wq.reshape(H, HD, DM).mean(axis=1, keepdims=True)).reshape(DM, DM)
    Wk = (Wk.reshape(H, HD, DM) -
          Wk.reshape(H, HD, DM).mean(axis=1, keepdims=True)).reshape(DM, DM)
    ln_g32 = np.ascontiguousarray(np.asarray(ln_g, dtype=np.float32))
    ln_b32 = np.ascontiguousarray(np.asarray(ln_b, dtype=np.float32))

    in_maps = []
    for c in range(NCORES):
        hs = slice(c * LOC, (c + 1) * LOC)
        in_maps.append({
            "x1": x1b,
            "x2": x2b,
            "wqT": np.ascontiguousarray(Wq[hs, :].T).astype(bf),
            "wkT": np.ascontiguousarray(Wk[hs, :].T).astype(bf),
            "wvT": np.ascontiguousarray(Wv[hs, :].T).astype(bf),
            "wp": np.ascontiguousarray(Wp[:, hs].T).astype(bf),
            "ln_g": ln_g32,
            "ln_b": ln_b32,
        })

    res = run_bass_kernel_spmd(nc, in_maps, core_ids=list(range(NCORES)))
    LAST_RESULT = res
    acc = np.zeros((B, N, DM), dtype=np.float32)
    for r in res.results:
        acc += np.asarray(r["out"], dtype=np.float32)
    acc += np.asarray(bp, dtype=np.float32)
    return acc

